# revision 7
# baseline (speedup 1.0000x reference)
import math
import numpy as np
from contextlib import ExitStack

import concourse.bass as bass
import concourse.tile as tile
from concourse import mybir, masks, bacc
from concourse.bass_utils import run_bass_kernel_spmd

f32, f32r, bf16 = mybir.dt.float32, mybir.dt.float32r, mybir.dt.bfloat16
ALU = mybir.AluOpType
AX = mybir.AxisListType
ACTF = mybir.ActivationFunctionType
MAGIC = float(2**23 + 2**22)

B, S, HID = 1, 2048, 2048
H, KVH, HD = 16, 4, 128
LEVEL = 256
NCORES = 8
HPC = H // NCORES            # q heads per core = 2
SC = 512                     # seq chunk for matmul free dim
NQT = S // 128               # q tiles = 16
NKC = S // SC                # k chunks = 4


def _s_eff(s, size, qp):
    # mimic reference fp32 arithmetic exactly: s_eff = |s|*g + (|s| - |s|*g)
    g = np.float32(1.0 / math.sqrt(size * qp))
    s = np.abs(np.float32(s))
    sg = np.float32(s * g)
    return float(np.float32(sg + np.float32(s - sg)))


def _build(causal, seq, sek, sev, sea, seo):
    nc = bacc.Bacc()
    hT = nc.declare_dram_parameter("hT", [HID, S], f32r, isOutput=False)
    wq = nc.declare_dram_parameter("wq", [HID, HPC * HD], f32r, isOutput=False)
    wk = nc.declare_dram_parameter("wk", [HID, HD], f32r, isOutput=False)
    wv = nc.declare_dram_parameter("wv", [HID, HD], f32r, isOutput=False)
    wo = nc.declare_dram_parameter("wo", [HPC * HD, HID], f32r, isOutput=False)
    ctq = nc.declare_dram_parameter("ctq", [HD, S], f32, isOutput=False)
    stq = nc.declare_dram_parameter("stq", [HD, S], f32, isOutput=False)
    ctk = nc.declare_dram_parameter("ctk", [HD, S], f32, isOutput=False)
    stk = nc.declare_dram_parameter("stk", [HD, S], f32, isOutput=False)
    if causal:
        bm = nc.declare_dram_parameter("bm", [4, 128, SC], f32, isOutput=False)
    else:
        bm = nc.declare_dram_parameter("bm", [S, S], f32, isOutput=False)
    outp = nc.declare_dram_parameter("outp", [S, HID], f32, isOutput=True)

    inv_sq = float(1.0 / seq)
    inv_sk = float(1.0 / sek)
    inv_sv = float(1.0 / sev)
    inv_sa = float(1.0 / sea)
    beta = float(np.float32(seq) * np.float32(sek) / np.float32(math.sqrt(HD)))
    alpha = float(np.float32(sea) * np.float32(sev) / np.float32(seo))

    with tile.TileContext(nc) as tc, ExitStack() as ctx:
        cp = ctx.enter_context(tc.tile_pool(name="const", bufs=1))
        live = ctx.enter_context(tc.tile_pool(name="live", bufs=1))

        # ---- constants / weights ----
        wq_sb = cp.tile([128, HID // 128, HPC * HD], f32r)
        nc.sync.dma_start(wq_sb[:], wq.rearrange("(c p) d -> p c d", p=128))
        wk_sb = cp.tile([128, HID // 128, HD], f32r)
        nc.sync.dma_start(wk_sb[:], wk.rearrange("(c p) d -> p c d", p=128))
        wv_sb = cp.tile([128, HID // 128, HD], f32r)
        nc.sync.dma_start(wv_sb[:], wv.rearrange("(c p) d -> p c d", p=128))
        wo_sb = cp.tile([128, HPC, HID], f32r)
        nc.sync.dma_start(wo_sb[:], wo.rearrange("(c p) n -> p c n", p=128))
        if causal:
            bm_sb = cp.tile([128, 4, SC], f32)
            nc.sync.dma_start(bm_sb[:], bm.rearrange("t p j -> p t j"))
        ident = cp.tile([128, 128], bf16)
        masks.make_identity(nc, ident[:])

        # ---- long-lived activations ----
        qr = [live.tile([128, S], f32r, tag=f"qr{h}", name=f"qr{h}") for h in range(HPC)]
        kr = live.tile([128, S], f32r)
        v_nat = live.tile([128, S], bf16)
        oT = [live.tile([128, S], f32r, tag=f"oT{h}", name=f"oT{h}") for h in range(HPC)]

        phase1 = ExitStack()
        ph = phase1.enter_context(tc.tile_pool(name="ph1", bufs=1))
        ctq_sb = ph.tile([128, S], f32)
        nc.sync.dma_start(ctq_sb[:], ctq[:])
        stq_sb = ph.tile([128, S], f32)
        nc.sync.dma_start(stq_sb[:], stq[:])
        ctk_sb = ph.tile([128, S], f32)
        nc.sync.dma_start(ctk_sb[:], ctk[:])
        stk_sb = ph.tile([128, S], f32)
        nc.sync.dma_start(stk_sb[:], stk[:])
        q_int = [ph.tile([128, S], f32, tag=f"qint{h}", name=f"qint{h}") for h in range(HPC)]
        k_int = ph.tile([128, S], f32)
        v_intT = ph.tile([128, S], bf16)

        tc.strict_bb_all_engine_barrier()

        # ================= projections =================
        def quant3(dst, src, inv_s, qn, qp, sc1, sc2):
            nc.vector.tensor_scalar(sc1[:], src, inv_s, float(qn),
                                    op0=ALU.mult, op1=ALU.max)
            nc.vector.tensor_scalar(sc2[:], sc1[:], float(qp), MAGIC,
                                    op0=ALU.min, op1=ALU.add)
            nc.vector.tensor_scalar(dst, sc2[:], MAGIC, None, op0=ALU.subtract)

        with tc.tile_pool(name="ht", bufs=3) as htp, \
             tc.tile_pool(name="pp", bufs=8, space="PSUM") as pp, \
             tc.tile_pool(name="qsc", bufs=2) as qsc:
            for half in range(2):
                ps = []
                for t in range(8):
                    ps.append(pp.tile([128, SC], f32, tag="projps", name=f"projps{t}"))
                for c in range(HID // 128):
                    ht_t = htp.tile([128, S // 2], f32r)
                    nc.sync.dma_start(
                        ht_t[:], hT[c * 128:(c + 1) * 128,
                                    half * (S // 2):(half + 1) * (S // 2)])
                    st, sp = (c == 0), (c == HID // 128 - 1)
                    for s2 in range(2):
                        rhs = ht_t[:, s2 * SC:(s2 + 1) * SC]
                        nc.tensor.matmul(ps[s2 * 4 + 0][:], lhsT=wq_sb[:, c, 0:128],
                                         rhs=rhs, start=st, stop=sp)
                        nc.tensor.matmul(ps[s2 * 4 + 1][:], lhsT=wq_sb[:, c, 128:256],
                                         rhs=rhs, start=st, stop=sp)
                        nc.tensor.matmul(ps[s2 * 4 + 2][:], lhsT=wk_sb[:, c, :],
                                         rhs=rhs, start=st, stop=sp)
                        nc.tensor.matmul(ps[s2 * 4 + 3][:], lhsT=wv_sb[:, c, :],
                                         rhs=rhs, start=st, stop=sp)
                for s2 in range(2):
                    lo = half * (S // 2) + s2 * SC
                    sl = slice(lo, lo + SC)
                    sc1 = qsc.tile([128, SC], f32, tag="sc1")
                    sc2 = qsc.tile([128, SC], f32, tag="sc2")
                    quant3(q_int[0][:, sl], ps[s2 * 4 + 0][:], inv_sq, -128, 127, sc1, sc2)
                    sc1 = qsc.tile([128, SC], f32, tag="sc1")
                    sc2 = qsc.tile([128, SC], f32, tag="sc2")
                    quant3(q_int[1][:, sl], ps[s2 * 4 + 1][:], inv_sq, -128, 127, sc1, sc2)
                    sc1 = qsc.tile([128, SC], f32, tag="sc1")
                    sc2 = qsc.tile([128, SC], f32, tag="sc2")
                    quant3(k_int[:, sl], ps[s2 * 4 + 2][:], inv_sk, -128, 127, sc1, sc2)
                    sc1 = qsc.tile([128, SC], f32, tag="sc1")
                    sc2 = qsc.tile([128, SC], f32, tag="sc2")
                    quant3(v_intT[:, sl], ps[s2 * 4 + 3][:], inv_sv, -128, 127, sc1, sc2)

        # ---- RoPE (scaled tables fold beta on q side) + V transpose ----
        with tc.tile_pool(name="rp", bufs=1) as rp, \
             tc.tile_pool(name="vt", bufs=2, space="PSUM") as vtp:
            def rope(dst, src, ct, st):
                sh = rp.tile([128, S], f32, tag="ropesh", name="ropesh")
                nc.sync.dma_start(sh[0:64, :], src[64:128, :])
                nc.sync.dma_start(sh[64:128, :], src[0:64, :])
                t1 = rp.tile([128, S], f32, tag="ropet1", name="ropet1")
                t2 = rp.tile([128, S], f32, tag="ropet2", name="ropet2")
                nc.vector.tensor_mul(t1[:], src[:], ct[:])
                nc.vector.tensor_mul(t2[:], sh[:], st[:])
                nc.vector.tensor_add(dst[:], t1[:], t2[:])

            for h in range(HPC):
                rope(qr[h], q_int[h], ctq_sb, stq_sb)
            rope(kr, k_int, ctk_sb, stk_sb)
            for kb in range(S // 128):
                vt = vtp.tile([128, 128], bf16, tag="vtp")
                nc.tensor.transpose(vt[:], v_intT[:, kb * 128:(kb + 1) * 128], ident[:])
                nc.vector.tensor_copy(v_nat[:, kb * 128:(kb + 1) * 128], vt[:])

        phase1.close()

        # ================= attention =================
        with tc.tile_pool(name="sps", bufs=1, space="PSUM") as sps, \
             tc.tile_pool(name="tps", bufs=2, space="PSUM") as tps, \
             tc.tile_pool(name="ops", bufs=2, space="PSUM") as ops, \
             tc.tile_pool(name="asc", bufs=1) as asc, \
             tc.tile_pool(name="at", bufs=1) as atp, \
             tc.tile_pool(name="mk", bufs=2) as mkp:
            for qc in range(NKC):
                nk = (qc + 1) if causal else NKC
                for h in range(HPC):
                    aT = atp.tile([128, 4 * nk, SC], bf16, tag="aT")
                    for qt in range(4):
                        qi = qc * 4 + qt
                        if not causal:
                            mrow = mkp.tile([128, S], f32, tag="mrow")
                            nc.sync.dma_start(mrow[:], bm[qi * 128:(qi + 1) * 128, :])
                        scp = sps.tile([128, nk * SC], f32, tag="scores")
                        for kc in range(nk):
                            nc.tensor.matmul(
                                scp[:, kc * SC:(kc + 1) * SC],
                                lhsT=qr[h][:, qi * 128:(qi + 1) * 128],
                                rhs=kr[:, kc * SC:(kc + 1) * SC],
                                start=True, stop=True)
                        if causal:
                            nc.vector.tensor_add(scp[:, (nk - 1) * SC:nk * SC],
                                                 scp[:, (nk - 1) * SC:nk * SC],
                                                 bm_sb[:, qt, :])
                        else:
                            nc.vector.tensor_add(scp[:], scp[:], mrow[:, 0:nk * SC])
                        negm = asc.tile([128, 1], f32, tag="negm")
                        nc.vector.tensor_reduce(negm[:], scp[:], op=ALU.max,
                                                axis=AX.X, negate=True)
                        ex = asc.tile([128, nk * SC], f32, tag="ex")
                        rs = asc.tile([128, 1], f32, tag="rs")
                        nc.scalar.activation(ex[:], scp[:], ACTF.Exp,
                                             bias=negm[:], scale=1.0, accum_out=rs[:])
                        inv0 = asc.tile([128, 1], f32, tag="inv0")
                        nc.vector.reciprocal(inv0[:], rs[:])
                        t = asc.tile([128, 1], f32, tag="nt")
                        nc.vector.tensor_mul(t[:], rs[:], inv0[:])
                        t2 = asc.tile([128, 1], f32, tag="nt2")
                        nc.vector.tensor_scalar(t2[:], t[:], -1.0, 2.0,
                                                op0=ALU.mult, op1=ALU.add)
                        inv1 = asc.tile([128, 1], f32, tag="inv1")
                        nc.vector.tensor_mul(inv1[:], inv0[:], t2[:])
                        crow = asc.tile([128, 1], f32, tag="crow")
                        nc.vector.tensor_scalar(crow[:], inv1[:], inv_sa, None, op0=ALU.mult)
                        aq1 = asc.tile([128, nk * SC], f32, tag="aq1")
                        nc.vector.tensor_scalar(aq1[:], ex[:], crow[:], MAGIC,
                                                op0=ALU.mult, op1=ALU.add)
                        a_int = asc.tile([128, nk * SC], bf16, tag="aint")
                        nc.vector.tensor_scalar(a_int[:], aq1[:], MAGIC, None,
                                                op0=ALU.subtract)
                        for kb in range(4 * nk):
                            tp = tps.tile([128, 128], bf16, tag="tp")
                            nc.tensor.transpose(
                                tp[:], a_int[:, kb * 128:(kb + 1) * 128], ident[:])
                            nc.vector.tensor_copy(
                                aT[:, kb, qt * 128:(qt + 1) * 128], tp[:])
                    po = ops.tile([128, SC], f32, tag="po")
                    for kb in range(4 * nk):
                        nc.tensor.matmul(po[:], lhsT=v_nat[:, kb * 128:(kb + 1) * 128],
                                         rhs=aT[:, kb, :],
                                         start=(kb == 0), stop=(kb == 4 * nk - 1))
                    sc1 = asc.tile([128, SC], f32, tag="osc1")
                    sc2 = asc.tile([128, SC], f32, tag="osc2")
                    nc.vector.tensor_scalar(sc1[:], po[:], alpha, 0.0,
                                            op0=ALU.mult, op1=ALU.max)
                    nc.vector.tensor_scalar(sc2[:], sc1[:], 255.0, MAGIC,
                                            op0=ALU.min, op1=ALU.add)
                    nc.vector.tensor_scalar(oT[h][:, qc * SC:(qc + 1) * SC], sc2[:],
                                            MAGIC, None, op0=ALU.subtract)

        # ================= output projection =================
        with tc.tile_pool(name="wps", bufs=4, space="PSUM") as wps, \
             tc.tile_pool(name="ost", bufs=2) as ostp:
            for st in range(NQT):
                stage = ostp.tile([128, HID], f32, tag="stage")
                for hc in range(HID // SC):
                    pw = wps.tile([128, SC], f32, tag="pw")
                    for h in range(HPC):
                        nc.tensor.matmul(pw[:], lhsT=oT[h][:, st * 128:(st + 1) * 128],
                                         rhs=wo_sb[:, h, hc * SC:(hc + 1) * SC],
                                         start=(h == 0), stop=(h == HPC - 1))
                    nc.vector.tensor_copy(stage[:, hc * SC:(hc + 1) * SC], pw[:])
                nc.sync.dma_start(outp[st * 128:(st + 1) * 128, :], stage[:])

    nc.finalize()
    return nc


_CACHE = {}


def kernel(**inputs):
    hidden = np.asarray(inputs["hidden_states"], np.float32)[0]      # [S, HID]
    Wq = np.asarray(inputs["Wq"], np.float32)
    Wk = np.asarray(inputs["Wk"], np.float32)
    Wv = np.asarray(inputs["Wv"], np.float32)
    Wo = np.asarray(inputs["Wo"], np.float32)
    cos = np.asarray(inputs["cos"], np.float32)[0]                   # [S, HD]
    sin = np.asarray(inputs["sin"], np.float32)[0]
    mask = np.asarray(inputs["attention_mask"], np.float32)[0, 0]    # [S, S]

    seq = _s_eff(inputs["s_q"][0], B * H * S * HD, LEVEL // 2 - 1)
    sek = _s_eff(inputs["s_k"][0], B * KVH * S * HD, LEVEL // 2 - 1)
    sev = _s_eff(inputs["s_v"][0], B * KVH * S * HD, LEVEL // 2 - 1)
    sea = _s_eff(inputs["s_attn"][0], B * H * S * S, LEVEL - 1)
    seo = _s_eff(inputs["s_out"][0], B * H * S * HD, LEVEL - 1)

    tri = np.tril(np.ones((S, S), bool))
    causal = bool((mask == np.where(tri, np.float32(0.0), np.float32(-1e9))).all())

    key = (causal, seq, sek, sev, sea, seo)
    if key not in _CACHE:
        _CACHE[key] = _build(*key)
    nc = _CACHE[key]

    beta = np.float32(seq) * np.float32(sek) / np.float32(math.sqrt(HD))
    sgn = np.concatenate([-np.ones(64, np.float32), np.ones(64, np.float32)])
    cT = np.ascontiguousarray(cos.T)                                  # [HD, S]
    sT = np.ascontiguousarray(sin.T) * sgn[:, None]
    hT = np.ascontiguousarray(hidden.T)

    if causal:
        bmn = np.zeros((4, 128, SC), np.float32)
        for t in range(4):
            r = np.arange(128)[:, None]
            j = np.arange(SC)[None, :]
            bmn[t] = np.where(j <= t * 128 + r, 0.0, -1e9).astype(np.float32)
    else:
        bmn = mask

    in_maps = []
    for c in range(NCORES):
        h0 = c * HPC
        g = h0 // (H // KVH)
        wo_c = np.ascontiguousarray(Wo[:, h0 * HD:(h0 + HPC) * HD].T) * np.float32(seo)
        in_maps.append({
            "hT": hT,
            "wq": np.ascontiguousarray(Wq[h0 * HD:(h0 + HPC) * HD, :].T),
            "wk": np.ascontiguousarray(Wk[g * HD:(g + 1) * HD, :].T),
            "wv": np.ascontiguousarray(Wv[g * HD:(g + 1) * HD, :].T),
            "wo": wo_c,
            "ctq": cT * beta, "stq": sT * beta,
            "ctk": cT, "stk": sT,
            "bm": bmn,
        })

    res = run_bass_kernel_spmd(nc, in_maps, list(range(NCORES)))
    out = np.zeros((S, HID), np.float64)
    for c in range(NCORES):
        out += res.results[c]["outp"].astype(np.float64)
    return out.astype(np.float32)[None]
